# revision 4
# baseline (speedup 1.0000x reference)
"""TimeSformer-style divided space-time attention block on 8 trn2 cores.

v2: fp8(e4m3) DoubleRow matmuls for the big GEMMs (qkv x2, proj x2, tfc,
fc1 hi/lo-split); fc2 stays bf16 for accuracy. Data-parallel over B=8,
one batch element per core, zero collectives. Feature-major activations
([C partitions, token free]); x transposed host-side.

Scales: fp8 weights carry x32; o/psb activations carry x4 (folded into
evacuation scales). fc1 = Whi + Wlo fp8 double-pass (residual split).
"""
import sys
import os

sys.path.insert(0, "/opt/trn_rl_repo")

import numpy as np
import ml_dtypes

import bass_rust
import concourse.bass as bass
import concourse.mybir as mybir
from concourse.tile import TileContext
import concourse.tile as tile_mod
from concourse.vector_clock import ScopedClock
from concourse.bass_utils import run_bass_kernel_spmd

F32 = mybir.dt.float32
BF16 = mybir.dt.bfloat16
F8 = mybir.dt.float8e4
DR = mybir.MatmulPerfMode.DoubleRow
AF = mybir.ActivationFunctionType
ALU = mybir.AluOpType
BF = ml_dtypes.bfloat16
NF8 = ml_dtypes.float8_e4m3
WSC = 32.0          # fp8 weight scale
OSC = 4.0           # fp8 activation scale for o / psb

C = 1024
KC = 8          # C / 128
HEADS = 16
D = 64
T = 16
HW = 256
NG = 4096       # grid tokens
SCALE = D ** -0.5
EPS = 1e-5
MLP = 4096

# --------------------------------------------------------------------------
# Workarounds for this walrus build's 1-wait-per-instruction cap.
_ws_ctr = [0]


def _patched_drain_and_barrier(self, tick_clock, wait_clock):
    nc = self.nc
    probe = nc.sync.nop()
    wait_clock.add_sem_waits(probe.ins, ScopedClock({None: tick_clock.global_clock}))
    waits = list(probe.ins.sync_info.on_wait) if probe.ins.sync_info else []
    chunks = [[w] for w in waits] or [[]]
    probe.ins.sync_info = bass_rust.SyncInfo(on_wait=chunks[0], on_update=[])
    for ch in chunks[1:]:
        n = nc.sync.nop()
        n.ins.sync_info = bass_rust.SyncInfo(on_wait=ch, on_update=[])
    nc.sync.drain()
    nc.all_engine_barrier()
    assert self.sems is not None
    popped = nc._tile_sem_poison_stack.pop()
    assert popped is self._sem_poison
    nc.clear_and_free_semaphores(list(self.sems.allocated().values()))
    nc.all_engine_barrier()


tile_mod.TileContext._drain_and_barrier = _patched_drain_and_barrier


def split_waits(nc, cap=1):
    for f in nc.m.functions:
        for bb in f.blocks:
            out = []
            changed = False
            for inst in bb.instructions:
                si = inst.sync_info
                waits = list(si.on_wait) if (si is not None and si.on_wait) else []
                if len(waits) > cap:
                    changed = True
                    extra, keep = waits[:-cap], waits[-cap:]
                    for w in extra:
                        _ws_ctr[0] += 1
                        nop = bass_rust.InstNoOp(
                            name=f"wsplit-{_ws_ctr[0]}", ins=[], outs=[])
                        nop.engine = inst.engine
                        nop.sync_info = bass_rust.SyncInfo(on_wait=[w], on_update=[])
                        out.append(nop)
                    inst.sync_info = bass_rust.SyncInfo(
                        on_wait=keep,
                        on_update=list(si.on_update) if si.on_update else [])
                out.append(inst)
            if changed:
                bb.instructions = out


def _bc(ap_slice, n):
    """free-dim step-0 broadcast of a [P, 1] slice to [P, n]."""
    return bass.AP(tensor=ap_slice.tensor, offset=ap_slice.offset,
                   ap=[list(ap_slice.ap[0]), [0, n]])


def _bc3(ap2, k):
    """insert a step-0 middle dim: [P, n] AP -> [P, k, n]."""
    return bass.AP(tensor=ap2.tensor, offset=ap2.offset,
                   ap=[list(ap2.ap[0]), [0, k], list(ap2.ap[1])])


# --------------------------------------------------------------------------
def build():
    nc = bass.Bass()
    io = dict(
        xfm=nc.dram_tensor("xfm", [C, NG + 1], F32, kind="ExternalInput"),
        # fp8 weights in [128, KC, M] layout (partition, k-tile, out-col)
        w_tqkv=nc.dram_tensor("w_tqkv", [128, KC, 3 * C], F8, kind="ExternalInput"),
        w_qkv=nc.dram_tensor("w_qkv", [128, KC, 3 * C], F8, kind="ExternalInput"),
        w_tproj=nc.dram_tensor("w_tproj", [128, KC, C], F8, kind="ExternalInput"),
        w_proj=nc.dram_tensor("w_proj", [128, KC, C], F8, kind="ExternalInput"),
        w_tfc=nc.dram_tensor("w_tfc", [128, KC, C], F8, kind="ExternalInput"),
        w_fc1h=nc.dram_tensor("w_fc1h", [128, KC, MLP], F8, kind="ExternalInput"),
        w_fc1l=nc.dram_tensor("w_fc1l", [128, KC, MLP], F8, kind="ExternalInput"),
        w_fc2=nc.dram_tensor("w_fc2", [MLP, C], BF16, kind="ExternalInput"),
        vecs=nc.dram_tensor("vecs", [C, 11], F32, kind="ExternalInput"),
        f1b=nc.dram_tensor("f1b", [MLP, 1], F32, kind="ExternalInput"),
        mask=nc.dram_tensor("mask", [128, 128], BF16, kind="ExternalInput"),
        out=nc.dram_tensor("out", [C, NG + 1], F32, kind="ExternalOutput"),
    )
    with TileContext(nc) as tc:
        _program(nc, tc, io)
    split_waits(nc)
    return nc


def _program(nc, tc, io):
    from contextlib import ExitStack
    mm = nc.tensor.matmul
    act = nc.scalar.activation
    dve = nc.vector

    ctx = ExitStack()
    with ctx:
        const = ctx.enter_context(tc.tile_pool(name="const", bufs=1))
        dram = ctx.enter_context(tc.tile_pool(name="dram", bufs=1, space="DRAM"))
        clsp = ctx.enter_context(tc.tile_pool(name="clsp", bufs=1))

        vec = const.tile([128, KC, 11], F32, tag="vecs", name="vecs")
        nc.sync.dma_start(vec, io["vecs"].rearrange("(k p) v -> p k v", p=128))
        f1b = const.tile([128, 32], F32, tag="f1b", name="f1b")
        nc.sync.dma_start(f1b, io["f1b"][:, 0].rearrange("(t p) -> p t", p=128))
        mask = const.tile([128, 128], BF16, tag="mask", name="mask")
        nc.sync.dma_start(mask, io["mask"][:, :])
        ones1 = const.tile([1, 128], BF16, tag="ones1", name="ones1")
        dve.memset(ones1, 1.0)
        onesK = const.tile([128, 1], BF16, tag="onesK", name="onesK")
        dve.memset(onesK, 1.0)
        # 0.25-valued rowsum lhsT: makes rc = 4/den so o comes out as 4*o
        # (fp8 activation scale OSC) with no extra ops.
        ones64 = const.tile([128, 64], BF16, tag="ones64", name="ones64")
        dve.memset(ones64, 1.0 / OSC)
        eps1 = const.tile([1, 1], F32, tag="eps1", name="eps1")
        dve.memset(eps1, EPS)

        def V(i):
            return dict(
                tng=vec[:, i, 0:1], tnb=vec[:, i, 1:2], n1g=vec[:, i, 2:3],
                n1b=vec[:, i, 3:4], n2g=vec[:, i, 4:5], n2b=vec[:, i, 5:6],
                tpb=vec[:, i, 6:7], pjb=vec[:, i, 7:8], tfb=vec[:, i, 8:9],
                f2b=vec[:, i, 9:10], zero=vec[:, i, 10:11])

        v_t = dram.tile([NG, C], BF16, tag="v_t", name="v_t")
        o_t = dram.tile([C, NG], F8, tag="o_t", name="o_t")
        xt = dram.tile([C, NG], BF16, tag="xt", name="xt")
        v_s = dram.tile([NG, C], BF16, tag="v_s", name="v_s")
        o_s = dram.tile([C, NG], F8, tag="o_s", name="o_s")
        xcat = dram.tile([C, NG + 1], BF16, tag="xcat", name="xcat")
        accd = dram.tile([C, NG + 1], F32, tag="accd", name="accd")

        xcls = clsp.tile([128, KC], F32, tag="xcls", name="xcls")
        xn_cls = clsp.tile([128, KC, 16], F8, tag="xncls", name="xncls")
        o_cls = clsp.tile([128, KC, T], F32, tag="ocls", name="ocls")
        vcls = clsp.tile([1, 1024], BF16, tag="vcls", name="vcls")

        # ---- shared LN helper --------------------------------------------
        def ln_chunk(sp, pp, src, isf, dst_write, n=512):
            """src: [128, KC, n] tile slice (f32 if isf else bf16)."""
            psum = pp.tile([1, 512], F32, tag="st_sum", name="st_sum")
            psq = pp.tile([1, 512], F32, tag="st_sq", name="st_sq")
            if isf:
                sbt = sp.tile([128, KC, 512], BF16, tag="lnb", name="lnb")
                act(sbt[:, :, 0:n], src, AF.Copy)
                sb = sbt[:, :, 0:n]
            else:
                sb = src
            sq = sp.tile([128, KC, 512], BF16, tag="lnq", name="lnq")
            dve.tensor_mul(sq[:, :, 0:n], sb, sb)
            for i in range(KC):
                mm(psum[:, 0:n], onesK, sb[:, i, :], start=(i == 0),
                   stop=(i == KC - 1), skip_group_check=True)
                mm(psq[:, 0:n], onesK, sq[:, i, 0:n], start=(i == 0),
                   stop=(i == KC - 1), skip_group_check=True)
            m_bf = sp.tile([1, 512], BF16, tag="st_mb", name="st_mb")
            act(m_bf[:, 0:n], psum[:, 0:n], AF.Copy, scale=1.0 / C)
            msq = sp.tile([1, 512], F32, tag="st_msq", name="st_msq")
            dve.tensor_mul(msq[:, 0:n], m_bf[:, 0:n], m_bf[:, 0:n])
            var = sp.tile([1, 512], F32, tag="st_var", name="st_var")
            dve.scalar_tensor_tensor(
                out=var[:, 0:n], in0=psq[:, 0:n], scalar=1.0 / C,
                in1=msq[:, 0:n], op0=ALU.mult, op1=ALU.subtract)
            sd = sp.tile([1, 512], F32, tag="st_sd", name="st_sd")
            act(sd[:, 0:n], var[:, 0:n], AF.Sqrt, bias=eps1)
            r_bf = sp.tile([1, 512], BF16, tag="st_rb", name="st_rb")
            with nc.allow_low_precision(reason="LN rstd consumed as bf16 anyway"):
                dve.reciprocal(r_bf[:, 0:n], sd[:, 0:n])
            pbc = pp.tile([128, 2, 512], F32, tag="st_bc", name="st_bc")
            mm(pbc[:, 0, 0:n], ones1, m_bf[:, 0:n], start=True, stop=True,
               skip_group_check=True)
            mm(pbc[:, 1, 0:n], ones1, r_bf[:, 0:n], start=True, stop=True,
               skip_group_check=True)
            t1 = sp.tile([128, KC, 512], F32, tag="ln_t1", name="ln_t1")
            dve.tensor_sub(t1[:, :, 0:n], sb, _bc3(pbc[:, 0, 0:n], KC))
            t2 = sp.tile([128, KC, 512], BF16, tag="ln_t2", name="ln_t2")
            dve.tensor_mul(t2[:, :, 0:n], t1[:, :, 0:n],
                           _bc3(pbc[:, 1, 0:n], KC))
            for i in range(KC):
                dst_write(i, t2[:, i, 0:n])

        def ln_cls_col(sp, pp, src_f32_or_bf, dst_write):
            """LN over the 1024 features of one [128, KC] column-packed token."""
            src, isf = src_f32_or_bf
            if isf:
                xb = sp.tile([128, KC], BF16, tag="clb", name="clb")
                act(xb, src, AF.Copy)
            else:
                xb = src
            xq = sp.tile([128, KC], BF16, tag="clq", name="clq")
            dve.tensor_mul(xq, xb, xb)
            pcs = pp.tile([1, 512], F32, tag="st_sum", name="st_sum")
            mm(pcs[:, 0:KC], onesK, xb, start=True, stop=True,
               skip_group_check=True)
            pcq = pp.tile([1, 512], F32, tag="st_sq", name="st_sq")
            mm(pcq[:, 0:KC], onesK, xq, start=True, stop=True,
               skip_group_check=True)
            cst = sp.tile([1, 8], F32, tag="clst", name="clst")
            dve.reduce_sum(cst[:, 0:1], pcs[:, 0:KC], axis=mybir.AxisListType.X)
            dve.reduce_sum(cst[:, 1:2], pcq[:, 0:KC], axis=mybir.AxisListType.X)
            act(cst[:, 2:3], cst[:, 0:1], AF.Copy, scale=1.0 / C)
            dve.tensor_mul(cst[:, 3:4], cst[:, 2:3], cst[:, 2:3])
            dve.scalar_tensor_tensor(
                out=cst[:, 4:5], in0=cst[:, 1:2], scalar=1.0 / C,
                in1=cst[:, 3:4], op0=ALU.mult, op1=ALU.subtract)
            act(cst[:, 5:6], cst[:, 4:5], AF.Sqrt, bias=eps1)
            dve.reciprocal(cst[:, 6:7], cst[:, 5:6])
            cmb = sp.tile([1, 2], BF16, tag="clmb", name="clmb")
            act(cmb[:, 0:1], cst[:, 2:3], AF.Copy)
            act(cmb[:, 1:2], cst[:, 6:7], AF.Copy)
            pbc = pp.tile([128, 2, 512], F32, tag="st_bc", name="st_bc")
            mm(pbc[:, 0, 0:1], ones1, cmb[:, 0:1], start=True, stop=True,
               skip_group_check=True)
            mm(pbc[:, 1, 0:1], ones1, cmb[:, 1:2], start=True, stop=True,
               skip_group_check=True)
            ct1 = sp.tile([128, KC], F32, tag="clt1", name="clt1")
            dve.tensor_sub(ct1, src if not isf else xb, _bc(pbc[:, 0, 0:1], KC))
            ct2 = sp.tile([128, KC], BF16, tag="clt2", name="clt2")
            dve.tensor_mul(ct2, ct1, _bc(pbc[:, 1, 0:1], KC))
            for i in range(KC):
                dst_write(i, ct2[:, i:i + 1])

        # ==================================================================
        # PHASE A + B: temporal LN, qkv, attention
        with tc.tile_pool(name="xnt", bufs=1) as xnt_pool:
            xn_t = xnt_pool.tile([128, KC, NG], F8, tag="xnt", name="xnt")
            with tc.tile_pool(name="pa", bufs=2) as sp, \
                 tc.tile_pool(name="pap", bufs=2, space="PSUM") as pp:
                for j in range(8):
                    xch = sp.tile([128, KC, 512], F32, tag="xa", name="xa")
                    nc.sync.dma_start(
                        xch, io["xfm"][:, j * 512:(j + 1) * 512]
                        .rearrange("(k p) n -> p k n", p=128))

                    def wr(i, t2, j=j):
                        act(xn_t[:, i, j * 512:(j + 1) * 512], t2, AF.Identity,
                            scale=V(i)["tng"], bias=V(i)["tnb"])
                    ln_chunk(sp, pp, xch, True, wr)

            # temporal V (token rows, grid order)
            with tc.tile_pool(name="pbw", bufs=1) as wp, \
                 tc.tile_pool(name="pb", bufs=3) as sp, \
                 tc.tile_pool(name="pbp", bufs=4, space="PSUM") as pp:
                wv = wp.tile([128, KC, 1024], F8, tag="wv", name="wv")
                nc.sync.dma_start(wv, io["w_tqkv"][:, :, 2048:3072])
                for tt in range(32):
                    for half in range(2):
                        pv = pp.tile([128, 512], F32, tag="pv", name="pv")
                        for kk in range(4):
                            mm(pv, xn_t[:, 2 * kk:2 * kk + 2,
                                        tt * 128:(tt + 1) * 128],
                               wv[:, 2 * kk:2 * kk + 2,
                                  half * 512:(half + 1) * 512],
                               start=(kk == 0), stop=(kk == 3), perf_mode=DR)
                        vst = sp.tile([128, 512], BF16, tag="vst", name="vst")
                        act(vst, pv, AF.Copy, scale=1.0 / WSC)
                        nc.sync.dma_start(
                            v_t[tt * 128:(tt + 1) * 128,
                                half * 512:(half + 1) * 512], vst)

            # temporal attention per head-pair
            with tc.tile_pool(name="pc2w", bufs=2) as wp, \
                 tc.tile_pool(name="pqk", bufs=2) as qkp, \
                 tc.tile_pool(name="pb2", bufs=3) as sp, \
                 tc.tile_pool(name="pbP", bufs=2, space="PSUM") as pp:
                for hp in range(8):
                    wqk = wp.tile([128, KC, 256], F8, tag="wqk", name="wqk",
                                  bufs=3)
                    nc.sync.dma_start(
                        wqk[:, :, 0:128],
                        io["w_tqkv"][:, :, hp * 128:(hp + 1) * 128])
                    nc.sync.dma_start(
                        wqk[:, :, 128:256],
                        io["w_tqkv"][:, :, 1024 + hp * 128:1024 + (hp + 1) * 128])
                    q_ev = qkp.tile([64, NG], BF16, tag="q_ev", name="q_ev")
                    q_od = qkp.tile([64, NG], BF16, tag="q_od", name="q_od")
                    k_ev = qkp.tile([64, NG], BF16, tag="k_ev", name="k_ev")
                    k_od = qkp.tile([64, NG], BF16, tag="k_od", name="k_od")
                    for j in range(8):
                        pq = pp.tile([128, 512], F32, tag="pqk", name="pq")
                        pk = pp.tile([128, 512], F32, tag="pqk", name="pk")
                        sl = slice(j * 512, (j + 1) * 512)
                        for kk in range(4):
                            mm(pq, wqk[:, 2 * kk:2 * kk + 2, 0:128],
                               xn_t[:, 2 * kk:2 * kk + 2, sl],
                               start=(kk == 0), stop=(kk == 3), perf_mode=DR)
                        for kk in range(4):
                            mm(pk, wqk[:, 2 * kk:2 * kk + 2, 128:256],
                               xn_t[:, 2 * kk:2 * kk + 2, sl],
                               start=(kk == 0), stop=(kk == 3), perf_mode=DR)
                        act(q_ev[:, sl], pq[0:64, :], AF.Copy, scale=1.0 / WSC)
                        act(q_od[:, sl], pq[64:128, :], AF.Copy, scale=1.0 / WSC)
                        act(k_ev[:, sl], pk[0:64, :], AF.Copy, scale=1.0 / WSC)
                        dve.scalar_tensor_tensor(
                            out=k_od[:, sl], in0=pk[64:128, :], scalar=1.0 / WSC,
                            in1=_bc(V(0)["zero"], 512)[0:64, :],
                            op0=ALU.mult, op1=ALU.add)
                    for g in range(16):
                        b0 = g * 2
                        vp = sp.tile([128, 2, 128], BF16, tag="vp", name="vp",
                                     bufs=6)
                        nc.sync.dma_start(
                            vp, v_t[b0 * 128:(b0 + 2) * 128,
                                    hp * 128:(hp + 1) * 128]
                            .rearrange("(a p) c -> p a c", p=128))
                        ps_s = pp.tile([128, 4, 128], F32, tag="ps_s", name="ps_s")
                        for bl in range(2):
                            bs = slice((b0 + bl) * 128, (b0 + bl + 1) * 128)
                            mm(ps_s[:, bl * 2, :], k_ev[:, bs], q_ev[:, bs],
                               start=True, stop=True)
                            mm(ps_s[:, bl * 2 + 1, :], k_od[:, bs],
                               q_od[:, bs], start=True, stop=True)
                        es = sp.tile([128, 4, 128], BF16, tag="es", name="es", bufs=4)
                        act(es, ps_s, AF.Exp, scale=SCALE)
                        esm = sp.tile([128, 4, 128], BF16, tag="esm", name="esm", bufs=4)
                        mbc = bass.AP(
                            tensor=mask.tensor, offset=mask.offset,
                            ap=[list(mask.ap[0]), [0, 4], list(mask.ap[1])])
                        dve.tensor_mul(esm, es, mbc)
                        ps_o = pp.tile([128, 2, 128], F32, tag="ps_o", name="ps_o")
                        ps_r = pp.tile([128, 2, 128], F32, tag="ps_r", name="ps_r")
                        for bl in range(2):
                            for h2 in range(2):
                                tpos = (0, 64 * h2)
                                rr = slice(64 * h2, 64 * h2 + 64)
                                mm(ps_o[rr, bl, :],
                                   vp[:, bl, h2 * 64:(h2 + 1) * 64],
                                   esm[:, bl * 2 + h2, :], start=True,
                                   stop=True, tile_position=tpos,
                                   skip_group_check=True)
                                mm(ps_r[rr, bl, :], ones64,
                                   esm[:, bl * 2 + h2, :], start=True,
                                   stop=True, tile_position=tpos,
                                   skip_group_check=True)
                        rc = sp.tile([128, 2, 128], F32, tag="rc", name="rc", bufs=4)
                        dve.reciprocal(rc, ps_r)
                        ost = sp.tile([128, 2, 128], F8, tag="ost", name="ost", bufs=4)
                        dve.tensor_mul(ost, ps_o, rc)
                        nc.sync.dma_start(
                            o_t[hp * 128:(hp + 1) * 128,
                                b0 * 128:(b0 + 2) * 128],
                            ost.rearrange("p a b -> p (a b)"))

        # ==================================================================
        # PHASE C: proj_t + tfc + residual -> xt
        with tc.tile_pool(name="pcw", bufs=1) as wp, \
             tc.tile_pool(name="pc", bufs=3) as sp, \
             tc.tile_pool(name="pcp", bufs=4, space="PSUM") as pp:
            wpj = wp.tile([128, KC, 1024], F8, tag="wpj", name="wpj")
            nc.sync.dma_start(wpj, io["w_tproj"][:, :, :])
            wtf = wp.tile([128, KC, 1024], F8, tag="wtf", name="wtf")
            nc.sync.dma_start(wtf, io["w_tfc"][:, :, :])
            for j in range(8):
                sl = slice(j * 512, (j + 1) * 512)
                och = sp.tile([128, KC, 512], F8, tag="oc", name="oc")
                nc.sync.dma_start(
                    och, o_t[:, sl].rearrange("(k p) n -> p k n", p=128))
                psb = sp.tile([128, KC, 512], F8, tag="psb", name="psb")
                for m in range(KC):
                    ps = pp.tile([128, 512], F32, tag="pjp", name="pjp")
                    for kk in range(4):
                        mm(ps, wpj[:, 2 * kk:2 * kk + 2,
                                   m * 128:(m + 1) * 128],
                           och[:, 2 * kk:2 * kk + 2, :],
                           start=(kk == 0), stop=(kk == 3), perf_mode=DR)
                    # psum = WSC*OSC*proj ; psb = OSC*(proj + tpb)
                    if m % 2 == 0:
                        act(psb[:, m, :], ps, AF.Identity, scale=1.0 / WSC,
                            bias=V(m)["tpb"])
                    else:
                        dve.scalar_tensor_tensor(
                            out=psb[:, m, :], in0=ps, scalar=1.0 / WSC,
                            in1=_bc(V(m)["tpb"], 512), op0=ALU.mult, op1=ALU.add)
                xrj = sp.tile([128, KC, 512], F32, tag="xrj", name="xrj")
                nc.sync.dma_start(
                    xrj, io["xfm"][:, sl].rearrange("(k p) n -> p k n", p=128))
                xts = sp.tile([128, KC, 512], BF16, tag="xts", name="xts")
                for m in range(KC):
                    ps = pp.tile([128, 512], F32, tag="ptf", name="ptf")
                    for kk in range(4):
                        mm(ps, wtf[:, 2 * kk:2 * kk + 2,
                                   m * 128:(m + 1) * 128],
                           psb[:, 2 * kk:2 * kk + 2, :],
                           start=(kk == 0), stop=(kk == 3), perf_mode=DR)
                    tr = sp.tile([128, 512], F32, tag="trs", name="trs")
                    if m % 2 == 1:
                        act(tr, ps, AF.Identity, scale=1.0 / (WSC * OSC),
                            bias=V(m)["tfb"])
                    else:
                        dve.scalar_tensor_tensor(
                            out=tr, in0=ps, scalar=1.0 / (WSC * OSC),
                            in1=_bc(V(m)["tfb"], 512), op0=ALU.mult, op1=ALU.add)
                    dve.tensor_add(xts[:, m, :], tr, xrj[:, m, :])
                nc.sync.dma_start(
                    xt[:, sl].rearrange("(k p) n -> p k n", p=128), xts)

        # ==================================================================
        # PHASE D + E: spatial LN, qkv, attention
        with tc.tile_pool(name="xns", bufs=1) as xns_pool:
            xn_s = xns_pool.tile([128, KC, NG], F8, tag="xns", name="xns")
            with tc.tile_pool(name="pd", bufs=2) as sp, \
                 tc.tile_pool(name="pdp", bufs=2, space="PSUM") as pp:
                nc.sync.dma_start(
                    xcls, io["xfm"][:, NG:NG + 1]
                    .rearrange("(k p) o -> p (k o)", p=128))

                def wrc(i, col):
                    act(xn_cls[:, i, 0:1], col, AF.Identity,
                        scale=V(i)["n1g"], bias=V(i)["n1b"])
                ln_cls_col(sp, pp, (xcls, True), wrc)

                for j in range(8):
                    xch = sp.tile([128, KC, 512], BF16, tag="xd", name="xd")
                    nc.sync.dma_start(
                        xch, xt[:, j * 512:(j + 1) * 512]
                        .rearrange("(k p) n -> p k n", p=128))

                    def wr(i, t2, j=j):
                        t3 = sp.tile([128, 512], F8, tag="xno", name="xno")
                        act(t3, t2, AF.Identity, scale=V(i)["n1g"],
                            bias=V(i)["n1b"])
                        # reorder grid (sl, f) -> frame-major f*256 + j*32 + sl
                        s2 = bass.AP(tensor=t3.tensor, offset=t3.offset,
                                     ap=[list(t3.ap[0]), [16, 32], [1, 16]])
                        dsl = xn_s[:, i, j * 32:]
                        dst = bass.AP(tensor=dsl.tensor, offset=dsl.offset,
                                      ap=[list(dsl.ap[0]), [1, 32], [256, 16]])
                        nc.gpsimd.tensor_copy(dst, s2)
                    ln_chunk(sp, pp, xch, False, wr)

            # spatial V
            with tc.tile_pool(name="pew", bufs=1) as wp, \
                 tc.tile_pool(name="pe0", bufs=3) as sp, \
                 tc.tile_pool(name="pe0p", bufs=2, space="PSUM") as pp:
                wv = wp.tile([128, KC, 1024], F8, tag="swv", name="swv")
                nc.sync.dma_start(wv, io["w_qkv"][:, :, 2048:3072])
                for tt in range(32):
                    for half in range(2):
                        pv = pp.tile([128, 512], F32, tag="spv", name="spv")
                        for kk in range(4):
                            mm(pv, xn_s[:, 2 * kk:2 * kk + 2,
                                        tt * 128:(tt + 1) * 128],
                               wv[:, 2 * kk:2 * kk + 2,
                                  half * 512:(half + 1) * 512],
                               start=(kk == 0), stop=(kk == 3), perf_mode=DR)
                        vst = sp.tile([128, 512], BF16, tag="svst", name="svst")
                        act(vst, pv, AF.Copy, scale=1.0 / WSC)
                        nc.sync.dma_start(
                            v_s[tt * 128:(tt + 1) * 128,
                                half * 512:(half + 1) * 512], vst)
                pvc = pp.tile([1, 2, 512], F32, tag="pvc", name="pvc")
                for half in range(2):
                    for i in range(KC):
                        mm(pvc[:, half, :], xn_cls[:, i, 0:1],
                           wv[:, i, half * 512:(half + 1) * 512],
                           start=(i == 0), stop=(i == KC - 1),
                           skip_group_check=True)
                act(vcls[:, 0:512], pvc[:, 0, :], AF.Copy, scale=1.0 / WSC)
                act(vcls[:, 512:1024], pvc[:, 1, :], AF.Copy, scale=1.0 / WSC)

            # spatial attention per head-pair
            with tc.tile_pool(name="pe1w", bufs=2) as wp, \
                 tc.tile_pool(name="peqk", bufs=2) as qkp, \
                 tc.tile_pool(name="pe1", bufs=3) as sp, \
                 tc.tile_pool(name="peP", bufs=2, space="PSUM") as pp:
                for hp in range(8):
                    wqk = wp.tile([128, KC, 256], F8, tag="swqk", name="swqk",
                                  bufs=3)
                    nc.sync.dma_start(
                        wqk[:, :, 0:128],
                        io["w_qkv"][:, :, hp * 128:(hp + 1) * 128])
                    nc.sync.dma_start(
                        wqk[:, :, 128:256],
                        io["w_qkv"][:, :, 1024 + hp * 128:1024 + (hp + 1) * 128])
                    q_ev = qkp.tile([64, NG], BF16, tag="sq_ev", name="sq_ev")
                    q_od = qkp.tile([64, NG], BF16, tag="sq_od", name="sq_od")
                    k_ev = qkp.tile([64, NG], BF16, tag="sk_ev", name="sk_ev")
                    k_od = qkp.tile([64, NG], BF16, tag="sk_od", name="sk_od")
                    qkc = qkp.tile([64, 4], BF16, tag="qkc", name="qkc")
                    for j in range(8):
                        pq = pp.tile([128, 512], F32, tag="spqk", name="spq")
                        pk = pp.tile([128, 512], F32, tag="spqk", name="spk")
                        sl = slice(j * 512, (j + 1) * 512)
                        for kk in range(4):
                            mm(pq, wqk[:, 2 * kk:2 * kk + 2, 0:128],
                               xn_s[:, 2 * kk:2 * kk + 2, sl],
                               start=(kk == 0), stop=(kk == 3), perf_mode=DR)
                        for kk in range(4):
                            mm(pk, wqk[:, 2 * kk:2 * kk + 2, 128:256],
                               xn_s[:, 2 * kk:2 * kk + 2, sl],
                               start=(kk == 0), stop=(kk == 3), perf_mode=DR)
                        act(q_ev[:, sl], pq[0:64, :], AF.Copy, scale=1.0 / WSC)
                        act(q_od[:, sl], pq[64:128, :], AF.Copy, scale=1.0 / WSC)
                        dve.scalar_tensor_tensor(
                            out=k_ev[:, sl], in0=pk[0:64, :], scalar=1.0 / WSC,
                            in1=_bc(V(0)["zero"], 512)[0:64, :],
                            op0=ALU.mult, op1=ALU.add)
                        dve.scalar_tensor_tensor(
                            out=k_od[:, sl], in0=pk[64:128, :], scalar=1.0 / WSC,
                            in1=_bc(V(0)["zero"], 512)[0:64, :],
                            op0=ALU.mult, op1=ALU.add)
                    pqc = pp.tile([128, 2], F32, tag="spqk", name="spqc")
                    for kk in range(4):
                        mm(pqc[:, 0:1], wqk[:, 2 * kk:2 * kk + 2, 0:128],
                           xn_cls[:, 2 * kk:2 * kk + 2, 0:1], start=(kk == 0),
                           stop=(kk == 3), perf_mode=DR, skip_group_check=True)
                    for kk in range(4):
                        mm(pqc[:, 1:2], wqk[:, 2 * kk:2 * kk + 2, 128:256],
                           xn_cls[:, 2 * kk:2 * kk + 2, 0:1], start=(kk == 0),
                           stop=(kk == 3), perf_mode=DR, skip_group_check=True)
                    act(qkc[:, 0:1], pqc[0:64, 0:1], AF.Copy, scale=1.0 / WSC)
                    act(qkc[:, 1:2], pqc[64:128, 0:1], AF.Copy, scale=1.0 / WSC)
                    act(qkc[:, 2:3], pqc[0:64, 1:2], AF.Copy, scale=1.0 / WSC)
                    act(qkc[:, 3:4], pqc[64:128, 1:2], AF.Copy, scale=1.0 / WSC)
                    # build q_ext [64, T, 257] = [cls | grid(f)]
                    qx = []
                    for h2 in range(2):
                        qsrc = q_ev if h2 == 0 else q_od
                        t = qkp.tile([64, T, 257], BF16, tag=f"qx{h2}", name=f"qx{h2}")
                        csl = qkc[:, h2:h2 + 1]
                        csrc = bass.AP(tensor=csl.tensor, offset=csl.offset,
                                       ap=[list(csl.ap[0]), [0, T], [1, 1]])
                        dve.tensor_copy(t[:, :, 0:1], csrc)
                        gsrc = bass.AP(tensor=qsrc.tensor, offset=qsrc.offset,
                                       ap=[list(qsrc.ap[0]), [256, T], [1, 256]])
                        nc.gpsimd.tensor_copy(t[:, :, 1:257], gsrc)
                        qx.append(t)
                    kcl = [qkc[:, 2:3], qkc[:, 3:4]]
                    for f in range(T):
                        vp = sp.tile([128, 2, 128], BF16, tag="svp", name="svp",
                                     bufs=6)
                        nc.sync.dma_start(
                            vp, v_s[2 * f * 128:(2 * f + 2) * 128,
                                    hp * 128:(hp + 1) * 128]
                            .rearrange("(a p) c -> p a c", p=128))
                        ps_o = pp.tile([128, 512], F32, tag="sps_o", name="sps_o", bufs=1)
                        ps_r = pp.tile([128, 512], F32, tag="sps_r", name="sps_r", bufs=1)
                        for ch in range(3):
                            ps_s = pp.tile([128, 2, 512], F32, tag="sps_s", name="sps_s", bufs=2)
                            es = sp.tile([128, 2, 512], BF16, tag="ses", name="ses", bufs=4)
                            for h2 in range(2):
                                ksrc = k_ev if h2 == 0 else k_od
                                qf = qx[h2][:, f, :]
                                if ch < 2:
                                    lh = ksrc[:, f * 256 + ch * 128:
                                              f * 256 + (ch + 1) * 128]
                                    mm(ps_s[:, h2, 0:257], lh, qf,
                                       start=True, stop=True)
                                else:
                                    mm(ps_s[0:1, h2, 0:257], kcl[h2], qf,
                                       start=True, stop=True)
                            pr_ = ps_s[:, :, 0:257] if ch < 2 \
                                else ps_s[0:1, :, 0:257]
                            eo = es[:, :, 0:257] if ch < 2 \
                                else es[0:1, :, 0:257]
                            act(eo, pr_, AF.Exp, scale=SCALE)
                            for h2 in range(2):
                                tpos = (0, 64 * h2)
                                rr = slice(64 * h2, 64 * h2 + 64)
                                if ch < 2:
                                    mm(ps_o[rr, 0:257],
                                       vp[:, ch, h2 * 64:(h2 + 1) * 64],
                                       es[:, h2, 0:257],
                                       start=(ch == 0), stop=False,
                                       tile_position=tpos,
                                       skip_group_check=True)
                                    mm(ps_r[rr, 0:257], ones64,
                                       es[:, h2, 0:257],
                                       start=(ch == 0), stop=False,
                                       tile_position=tpos,
                                       skip_group_check=True)
                                else:
                                    mm(ps_o[rr, 0:257],
                                       vcls[:, hp * 128 + h2 * 64:
                                            hp * 128 + (h2 + 1) * 64],
                                       es[0:1, h2, 0:257],
                                       start=False, stop=True,
                                       tile_position=tpos,
                                       skip_group_check=True)
                                    mm(ps_r[rr, 0:257], ones64[0:1, :],
                                       es[0:1, h2, 0:257],
                                       start=False, stop=True,
                                       tile_position=tpos,
                                       skip_group_check=True)
                        rc = sp.tile([128, 512], F32, tag="src", name="src", bufs=4)
                        dve.reciprocal(rc[:, 0:257], ps_r[:, 0:257])
                        ogr = sp.tile([128, 256], F8, tag="sogr", name="sogr", bufs=4)
                        dve.tensor_mul(ogr, ps_o[:, 1:257], rc[:, 1:257])
                        nc.sync.dma_start(
                            o_s[hp * 128:(hp + 1) * 128,
                                f * 256:(f + 1) * 256], ogr)
                        dve.tensor_mul(o_cls[:, hp, f:f + 1],
                                       ps_o[:, 0:1], rc[:, 0:1])

        # ==================================================================
        # PHASE F: proj_s + cls_t + xcat
        with tc.tile_pool(name="pfw", bufs=1) as wp, \
             tc.tile_pool(name="pfx", bufs=1) as xp, \
             tc.tile_pool(name="pf", bufs=3) as sp, \
             tc.tile_pool(name="pfp", bufs=3, space="PSUM") as pp:
            wps = wp.tile([128, KC, 1024], F8, tag="wps", name="wps")
            nc.sync.dma_start(wps, io["w_proj"][:, :, :])
            xtsb = xp.tile([128, KC, NG], BF16, tag="xtf", name="xtf")
            nc.sync.dma_start(xtsb, xt[:, :].rearrange("(k p) n -> p k n", p=128))
            ocb = sp.tile([128, KC, T], F8, tag="ocb", name="ocb")
            act(ocb, o_cls, AF.Copy)
            for j in range(8):
                sl = slice(j * 512, (j + 1) * 512)
                och = sp.tile([128, KC, 512], F8, tag="soc", name="soc")
                nc.sync.dma_start(
                    och, o_s[:, sl].rearrange("(k p) n -> p k n", p=128))
                for m in range(KC):
                    ps = pp.tile([128, 512], F32, tag="sfp", name="sfp")
                    for kk in range(4):
                        mm(ps, wps[:, 2 * kk:2 * kk + 2,
                                   m * 128:(m + 1) * 128],
                           och[:, 2 * kk:2 * kk + 2, :],
                           start=(kk == 0), stop=(kk == 3), perf_mode=DR)
                    res = sp.tile([128, 512], F32, tag="sres", name="sres")
                    if m % 2 == 0:
                        act(res, ps, AF.Identity, scale=1.0 / (WSC * OSC),
                            bias=V(m)["pjb"])
                    else:
                        dve.scalar_tensor_tensor(
                            out=res, in0=ps, scalar=1.0 / (WSC * OSC),
                            in1=_bc(V(m)["pjb"], 512), op0=ALU.mult, op1=ALU.add)
                    xs = xtsb[:, m, 2 * j:]
                    xap = bass.AP(tensor=xs.tensor, offset=xs.offset,
                                  ap=[list(xs.ap[0]), [1, 2], [16, 256]])
                    rap = bass.AP(tensor=res.tensor, offset=res.offset,
                                  ap=[list(res.ap[0]), [256, 2], [1, 256]])
                    xcs = sp.tile([128, 2, 256], BF16, tag="xcs", name="xcs")
                    dve.tensor_add(xcs, rap, xap)
                    nc.sync.dma_start(
                        xcat[m * 128:(m + 1) * 128, sl],
                        xcs.rearrange("p a b -> p (a b)"))
            for m in range(KC):
                ps = pp.tile([128, 512], F32, tag="scp", name="scp")
                for kk in range(4):
                    mm(ps[:, 0:T], wps[:, 2 * kk:2 * kk + 2,
                                       m * 128:(m + 1) * 128],
                       ocb[:, 2 * kk:2 * kk + 2, :], start=(kk == 0),
                       stop=(kk == 3), perf_mode=DR, skip_group_check=True)
                cres = sp.tile([128, T], F32, tag="cres", name="cres")
                act(cres, ps[:, 0:T], AF.Identity, scale=1.0 / (WSC * OSC),
                    bias=V(m)["pjb"])
                cm = sp.tile([128, 1], F32, tag="cm", name="cm")
                dve.reduce_sum(cm, cres, axis=mybir.AxisListType.X)
                cmx = sp.tile([128, 1], F32, tag="cmx", name="cmx")
                dve.scalar_tensor_tensor(
                    out=cmx, in0=cm, scalar=1.0 / T, in1=xcls[:, m:m + 1],
                    op0=ALU.mult, op1=ALU.add)
                cbf = sp.tile([128, 1], BF16, tag="cbf", name="cbf")
                act(cbf, cmx, AF.Copy)
                nc.sync.dma_start(xcat[m * 128:(m + 1) * 128, NG:NG + 1], cbf)

        # ==================================================================
        # PHASE G: MLP
        with tc.tile_pool(name="xn2p", bufs=1) as xn2_pool, \
             tc.tile_pool(name="pgw", bufs=1) as wp, \
             tc.tile_pool(name="pg", bufs=1) as lsp, \
             tc.tile_pool(name="pgp", bufs=1, space="PSUM") as lpp, \
             tc.tile_pool(name="pg2", bufs=2) as sp, \
             tc.tile_pool(name="pg2p", bufs=2, space="PSUM") as pp:
            xn2 = xn2_pool.tile([128, KC, 4112], F8, tag="xn2", name="xn2")

            def ln_j(j):
                xch = lsp.tile([128, KC, 512], BF16, tag="xg", name="xg")
                nc.sync.dma_start(
                    xch, xcat[:, j * 512:(j + 1) * 512]
                    .rearrange("(k p) n -> p k n", p=128))

                def wr(i, t2, j=j):
                    act(xn2[:, i, j * 512:(j + 1) * 512], t2, AF.Identity,
                        scale=V(i)["n2g"], bias=V(i)["n2b"])
                ln_chunk(lsp, lpp, xch, False, wr)

            def ln_cls():
                xcc = lsp.tile([128, KC], BF16, tag="xcc", name="xcc")
                nc.sync.dma_start(
                    xcc, xcat[:, NG:NG + 1].rearrange("(k p) o -> p (k o)",
                                                      p=128))

                def wrc2(i, col):
                    act(xn2[:, i, NG:NG + 1], col, AF.Identity,
                        scale=V(i)["n2g"], bias=V(i)["n2b"])
                ln_cls_col(lsp, lpp, (xcc, False), wrc2)

            def load_w(half):
                hsl = slice(half * 2048, (half + 1) * 2048)
                wf1h = wp.tile([128, KC, 2048], F8, tag="wf1h", name="wf1h")
                nc.sync.dma_start(wf1h, io["w_fc1h"][:, :, hsl])
                wf1l = wp.tile([128, KC, 2048], F8, tag="wf1l", name="wf1l")
                nc.sync.dma_start(wf1l, io["w_fc1l"][:, :, hsl])
                wf2 = wp.tile([128, 16, 1024], BF16, tag="wf2", name="wf2")
                nc.sync.dma_start(
                    wf2, io["w_fc2"][half * 2048:(half + 1) * 2048, :]
                    .rearrange("(t p) c -> p t c", p=128))
                return wf1h, wf1l, wf2

            N0S = [(0, 512), (512, 512), (1024, 512),
                   (1536, 512), (2048, 512), (2560, 512),
                   (3072, 512), (3584, 256), (3840, 257)]

            for half in range(2):
                wf1h, wf1l, wf2 = load_w(half)
                if half == 0:
                    schedule = []
                    for j in range(7):
                        schedule.append(("ln", j))
                        schedule.append(("mlp", N0S[j]))
                    schedule += [("ln", 7), ("mlp", N0S[7]), ("cls", None),
                                 ("mlp", N0S[8])]
                else:
                    schedule = [("mlp", c) for c in N0S]
                for kind, arg in schedule:
                    if kind == "ln":
                        ln_j(arg)
                        continue
                    if kind == "cls":
                        ln_cls()
                        continue
                    n0, nn = arg
                    if True:
                        hsb = []
                        for m in range(16):
                            pf1 = pp.tile([128, 512], F32, tag="pf1", name="pf1", bufs=2)
                            for kk in range(4):
                                mm(pf1[:, 0:nn],
                                   wf1h[:, 2 * kk:2 * kk + 2,
                                        m * 128:(m + 1) * 128],
                                   xn2[:, 2 * kk:2 * kk + 2, n0:n0 + nn],
                                   start=(kk == 0), stop=False, perf_mode=DR)
                            for kk in range(4):
                                mm(pf1[:, 0:nn],
                                   wf1l[:, 2 * kk:2 * kk + 2,
                                        m * 128:(m + 1) * 128],
                                   xn2[:, 2 * kk:2 * kk + 2, n0:n0 + nn],
                                   start=False, stop=(kk == 3), perf_mode=DR)
                            h = sp.tile([128, 512], BF16, tag=f"h{m}", name=f"h{m}")
                            act(h[:, 0:nn], pf1[:, 0:nn], AF.Gelu,
                                scale=1.0 / WSC,
                                bias=f1b[:, half * 16 + m:half * 16 + m + 1])
                            hsb.append(h)
                        for mo in range(KC):
                            pf2 = pp.tile([128, 512], F32, tag="pf2", name="pf2", bufs=2)
                            for k in range(16):
                                mm(pf2[:, 0:nn],
                                   wf2[:, k, mo * 128:(mo + 1) * 128],
                                   hsb[k][:, 0:nn], start=(k == 0),
                                   stop=(k == 15))
                            row = slice(mo * 128, (mo + 1) * 128)
                            if half == 0:
                                st = sp.tile([128, 512], F32, tag="ac0", name="ac0")
                                act(st[:, 0:nn], pf2[:, 0:nn], AF.Copy)
                                nc.sync.dma_start(accd[row, n0:n0 + nn],
                                                  st[:, 0:nn])
                            else:
                                t1 = sp.tile([128, 512], F32, tag="gf_t1", name="gf_t1")
                                act(t1[:, 0:nn], pf2[:, 0:nn], AF.Identity,
                                    bias=V(mo)["f2b"])
                                a0 = sp.tile([128, 512], F32, tag="gf_a0", name="gf_a0")
                                nc.sync.dma_start(a0[:, 0:nn],
                                                  accd[row, n0:n0 + nn])
                                xc = sp.tile([128, 512], BF16, tag="gf_xc", name="gf_xc")
                                nc.sync.dma_start(xc[:, 0:nn],
                                                  xcat[row, n0:n0 + nn])
                                s1 = sp.tile([128, 512], F32, tag="gf_s1", name="gf_s1")
                                dve.tensor_add(s1[:, 0:nn], t1[:, 0:nn],
                                               a0[:, 0:nn])
                                s2 = sp.tile([128, 512], F32, tag="gf_s2", name="gf_s2")
                                dve.tensor_add(s2[:, 0:nn], s1[:, 0:nn],
                                               xc[:, 0:nn])
                                nc.sync.dma_start(io["out"][row, n0:n0 + nn],
                                                  s2[:, 0:nn])


# --------------------------------------------------------------------------
_cache = {}


def _pack_w8(w, scale=WSC):
    """[M, C] torch Linear weight -> [128, KC, M] fp8 (x scale)."""
    wt = np.ascontiguousarray(np.asarray(w, np.float32).T) * scale  # [C, M]
    M = wt.shape[1]
    arr = wt.reshape(KC, 128, M).transpose(1, 0, 2)
    return np.ascontiguousarray(arr).astype(NF8)


def kernel(**inputs):
    x = np.asarray(inputs["x"], dtype=np.float32)        # [8, 4097, 1024]
    Bn = x.shape[0]

    w_tqkv = _pack_w8(inputs["tqkv_w"])
    w_qkv = _pack_w8(inputs["qkv_w"])
    w_tproj = _pack_w8(inputs["tproj_w"])
    w_proj = _pack_w8(inputs["proj_w"])
    w_tfc = _pack_w8(inputs["tfc_w"])
    # fc1 hi/lo split at the same x32 scale (lo catches hi's rounding error)
    wt1 = np.ascontiguousarray(np.asarray(inputs["fc1_w"], np.float32).T) * WSC
    hi = wt1.astype(NF8)
    lo = (wt1 - hi.astype(np.float32)).astype(NF8)
    w_fc1h = np.ascontiguousarray(
        hi.astype(np.float32).reshape(KC, 128, MLP).transpose(1, 0, 2)).astype(NF8)
    w_fc1l = np.ascontiguousarray(
        lo.astype(np.float32).reshape(KC, 128, MLP).transpose(1, 0, 2)).astype(NF8)
    w_fc2 = np.ascontiguousarray(
        np.asarray(inputs["fc2_w"], np.float32).T).astype(BF)

    vecs = np.stack([
        np.asarray(inputs["tnorm_g"]), np.asarray(inputs["tnorm_b"]),
        np.asarray(inputs["norm1_g"]), np.asarray(inputs["norm1_b"]),
        np.asarray(inputs["norm2_g"]), np.asarray(inputs["norm2_b"]),
        OSC * np.asarray(inputs["tproj_b"]), np.asarray(inputs["proj_b"]),
        np.asarray(inputs["tfc_b"]), np.asarray(inputs["fc2_b"]),
        np.zeros(C, np.float32)], axis=1).astype(np.float32)
    f1b = np.asarray(inputs["fc1_b"], dtype=np.float32).reshape(MLP, 1)
    mask = np.zeros((128, 128), np.float32)
    for s in range(8):
        mask[s * 16:(s + 1) * 16, s * 16:(s + 1) * 16] = 1.0
    mask = mask.astype(BF)

    if "nc" not in _cache:
        _cache["nc"] = build()
    nc = _cache["nc"]

    in_maps = []
    for b in range(Bn):
        xb = x[b]
        xfm = np.concatenate([xb[1:].T, xb[0:1].T], axis=1)
        in_maps.append(dict(
            xfm=np.ascontiguousarray(xfm), w_tqkv=w_tqkv, w_qkv=w_qkv,
            w_tproj=w_tproj, w_proj=w_proj, w_tfc=w_tfc, w_fc1h=w_fc1h,
            w_fc1l=w_fc1l, w_fc2=w_fc2, vecs=vecs, f1b=f1b, mask=mask))

    res = run_bass_kernel_spmd(nc, in_maps, core_ids=list(range(Bn)),
                               trace=os.environ.get("KTRACE", "0") == "1")
    if os.environ.get("KTRACE", "0") == "1" and res.exec_time_ns:
        print(f"HW exec time: {res.exec_time_ns} ns")

    out = np.empty((Bn, NG + 1, C), np.float32)
    for b in range(Bn):
        ofm = res.results[b]["out"]
        out[b, 0] = ofm[:, NG]
        grid = ofm[:, 0:NG].T.reshape(T, HW, C).transpose(1, 0, 2).reshape(NG, C)
        out[b, 1:] = grid
    return out
